# revision 20
# baseline (speedup 1.0000x reference)
"""Dense mean-field CRF (2-label Potts, gaussian + bilateral pairwise) on 8
Trainium2 NeuronCores.

Math: the bilateral kernel factorizes as S_spatial (separable, sigma=50) o
B_intensity (gaussian gram on pixel values). B ~= P @ P.T (Nystrom over 256
landmark intensities) at rank 28, so each mean-field message is 28 separable
bilateral channels plus one sigma=3 gaussian channel:

    msg = sum_ch praw_ch o (R_ch (p10_ch o h) R_ch),  R = S or G per channel
    logit = cb + msg - 13*h,   h = tanh(logit/2)

(the self-exclusion and Potts row terms collapse into the -13h and cb.)

Distribution: fully replicated — every core computes the identical full
problem, so there are no collectives, no cross-core rendezvous, and the
measured span is single-core latency. Per iteration the engines split:
PE runs 58 per-rank 96x96x96 f32 matmuls (exact f32 mandatory: any reduced-
precision matmul mode — f32r, bf16, even bf16 in the last iteration only —
flips argmax pixels after error amplification through the 5 iterations),
ACT does the PSUM->SBUF stage copies + tanh, DVE does the two elementwise
muls and per-chunk fold-to-96 tree reductions that overlap the PE burst.
Iteration 1's p10 o h0 is a host constant (h0 = tanh(cb/2) depends only on
the mask), so it runs without the wp multiply while the input DMAs stream
in (few, wide DMAs: each dma_start costs ~600ns of SP-sequencer config).
A back-to-back dummy-matmul stream covers the inter-iteration PE-idle
window so the HAM clock-gate sees sustained activity and holds the PE at
2.4 GHz for the whole kernel; without it each iteration restarts at half
clock for ~3-5us. The gaussian channel's stage A reads h directly, giving
the PE real work the moment tanh lands, and the first bilateral chunk is
narrow (4 channels) so its wp multiply unblocks stage A quickly.
"""
import sys
sys.path.insert(0, '/opt/trn_rl_repo')
import numpy as np

H = W = 96
KRANK = 28               # bilateral Nystrom rank
CHUNKS = [4, 8, 8, 8]    # bilateral channels per chunk: small first chunk
NCHUNK = len(CHUNKS)     # shortens the DVE fill-bubble at iteration start
COFF = [0, 4, 12, 20]    # channel offset per chunk
KW = KRANK * 96          # 2688
NITER = 5
NCORES = 8
EPS = 1e-8

_CACHE = {}
LAST_RESULTS = None


# ------------------------- host precomputation -------------------------

def _nystrom_P(f64, krank=KRANK):
    """Rank-k factor P [N, k] with exp(-(fi-fj)^2/400) ~= P @ P.T"""
    t = np.linspace(f64.min() - 1.0, f64.max() + 1.0, 256)
    Ktt = np.exp(-(t[:, None] - t[None, :]) ** 2 / 400.0)
    Kft = np.exp(-(f64[:, None] - t[None, :]) ** 2 / 400.0)
    lam, V = np.linalg.eigh(Ktt)
    keep = lam > lam.max() * 1e-14
    R = V[:, keep] / np.sqrt(lam[keep])
    Praw = Kft @ R
    mu, Wv = np.linalg.eigh(Praw.T @ Praw)
    idx = np.argsort(mu)[::-1][:krank]
    return Praw @ Wv[:, idx]          # float64 [N, krank]


def _host_constants(image, mask):
    img64 = np.asarray(image, dtype=np.float64).reshape(H, W)
    m = np.asarray(mask).reshape(-1)
    f64 = img64.reshape(-1)

    P3 = _nystrom_P(f64).reshape(H, W, KRANK)
    b = np.where(m == 0, np.log(EPS), -np.log(EPS)).reshape(H, W)
    h0 = np.tanh(b / 2.0)

    # channel-major [y, (ch, x)], bilateral channels only
    praw = np.ascontiguousarray(np.transpose(P3, (0, 2, 1)))
    wp1 = np.ascontiguousarray(np.transpose(10.0 * P3, (0, 2, 1))
                               * h0[:, None, :])

    idx = np.arange(96, dtype=np.float64)
    d2 = (idx[:, None] - idx[None, :]) ** 2

    to32 = lambda a: np.ascontiguousarray(a, dtype=np.float32)
    sml = np.concatenate([np.exp(-d2 / 5000.0), np.exp(-d2 / 18.0),
                          h0, b], axis=1)
    return {
        "sml": to32(sml),                      # s1 | g1 | h0 | cb
        "wp1": to32(wp1.reshape(H, KW)),
        "praw": to32(praw.reshape(H, KW)),
    }


# ------------------------- device program -------------------------

def _build():
    import concourse.bacc as bacc
    import concourse.mybir as mybir
    import concourse.tile as tile

    F32 = mybir.dt.float32
    AF = mybir.ActivationFunctionType
    ALU = mybir.AluOpType

    nc = bacc.Bacc("TRN2", target_bir_lowering=False, debug=False,
                   num_devices=NCORES)

    t_sml = nc.dram_tensor("sml", [96, 4 * 96], F32, kind="ExternalInput")
    t_wp1 = nc.dram_tensor("wp1", [96, KW], F32, kind="ExternalInput")
    t_praw = nc.dram_tensor("praw", [96, KW], F32, kind="ExternalInput")
    out_t = nc.dram_tensor("logit_out", [96, 96], F32, kind="ExternalOutput")

    with tile.TileContext(nc) as tc:
        with (
            tc.tile_pool(name="const", bufs=1) as cpool,
            tc.tile_pool(name="work", bufs=2) as wpool,
            tc.tile_pool(name="psA", bufs=2, space="PSUM") as psA,
            tc.tile_pool(name="psM", bufs=2, space="PSUM") as psM,
        ):
            # every dma_start costs ~600ns of SP-sequencer config before
            # any transfer begins: pack inputs into few DMAs, ordered by
            # first use (wp1 chunk 0 unblocks iteration 1 immediately).
            sb = {}
            sml = cpool.tile([96, 4 * 96], F32, tag="sml")
            nc.sync.dma_start(sml[:], t_sml[:])
            sb["s1"] = sml[:, 0:96]
            sb["g1"] = sml[:, 96:192]
            sb["h0"] = sml[:, 192:288]
            sb["cb"] = sml[:, 288:384]
            wp1sb = cpool.tile([96, KW], F32, tag="wp1")
            prawsb = cpool.tile([96, KW], F32, tag="praw")
            sb["wp1"] = wp1sb[:]
            sb["praw"] = prawsb[:]
            nc.sync.dma_start(wp1sb[:, 0:384], t_wp1[:, 0:384])
            nc.sync.dma_start(wp1sb[:, 384:1536], t_wp1[:, 384:1536])
            nc.sync.dma_start(prawsb[:, 0:1344], t_praw[:, 0:1344])
            nc.sync.dma_start(wp1sb[:, 1536:KW], t_wp1[:, 1536:KW])
            nc.sync.dma_start(prawsb[:, 1344:KW], t_praw[:, 1344:KW])

            # p10 derived on device (saves 1MB of ramp DMA)
            p10 = cpool.tile([96, KW], F32, tag="p10")
            nc.vector.tensor_scalar_mul(p10[:], sb["praw"][:], 10.0)

            def rview(ap, r):
                return ap.rearrange("p (r x) -> p r x", r=r)

            def pview(ap, r):
                return ap.rearrange("p (r z) -> p r z", r=r)[:, :, 0:96]

            def dummies(n, name):
                for j in range(n):
                    warm = psA.tile([96, 8 * 128], F32, tag="ptA",
                                    name=f"{name}_{j}")
                    nc.tensor.matmul(warm[:, 0:96], sb["s1"][:],
                                     sb["s1"][:], start=True, stop=True)

            # wp1 chunk 0 lands before the PE sequencer finishes booting,
            # so iteration 1 starts immediately; a couple of dummies just
            # ahead of it start the HAM activity window early.
            dummies(2, "prewarm")

            h = sb["h0"]
            for it in range(NITER):
                # gaussian micro-chunk first: stage A reads h directly, so
                # the PE has real work the moment tanh lands
                ptAg = psA.tile([96, 8 * 128], F32, tag="ptA",
                                name=f"Ag{it}")
                nc.tensor.matmul(ptAg[:, 0:96], h[:], sb["g1"][:],
                                 start=True, stop=True)

                # bilateral wp multiplies (DVE strict FIFO: all first)
                wpcs = []
                for c in range(NCHUNK):
                    w = CHUNKS[c] * 96
                    o = COFF[c] * 96
                    if it == 0:
                        wpcs.append(sb["wp1"][:, o:o + w])
                    else:
                        wpt = wpool.tile([96, w], F32, tag=f"wp{c}",
                                         name=f"wp{it}_{c}")
                        nc.vector.tensor_mul(
                            rview(wpt[:], CHUNKS[c]),
                            rview(p10[:, o:o + w], CHUNKS[c]),
                            h[:].unsqueeze(1).broadcast_to(
                                [96, CHUNKS[c], 96]))
                        wpcs.append(wpt[:])

                # base = cb - 13h (off the critical path)
                basev = wpool.tile([96, 96], F32, tag="base",
                                   name=f"base{it}")
                nc.vector.scalar_tensor_tensor(
                    basev[:], h[:], -13.0, sb["cb"][:],
                    op0=ALU.mult, op1=ALU.add)

                tsg = wpool.tile([96, 96], F32, tag="tsg", name=f"tsg{it}")
                nc.scalar.activation(tsg[:], ptAg[:, 0:96], AF.Copy)

                ptAs, tss, ptMs, mms, qs = {}, {}, {}, {}, {}

                def emit_A(c):
                    ptA = psA.tile([96, 8 * 128], F32, tag="ptA",
                                   name=f"A{it}_{c}")
                    for r in range(CHUNKS[c]):
                        nc.tensor.matmul(ptA[:, r * 128:r * 128 + 96],
                                         wpcs[c][:, r * 96:(r + 1) * 96],
                                         sb["s1"][:], start=True, stop=True)
                    ptAs[c] = ptA

                def emit_ts(c):
                    ts = wpool.tile([96, CHUNKS[c] * 96], F32, tag=f"ts{c}",
                                    name=f"ts{it}_{c}")
                    nc.scalar.activation(
                        rview(ts[:], CHUNKS[c]),
                        pview(ptAs[c][:, :CHUNKS[c] * 128], CHUNKS[c]),
                        AF.Copy)
                    tss[c] = ts

                def emit_B(c):
                    ptM = psM.tile([96, 8 * 128], F32, tag="ptM",
                                   name=f"M{it}_{c}")
                    for r in range(CHUNKS[c]):
                        nc.tensor.matmul(ptM[:, r * 128:r * 128 + 96],
                                         tss[c][:, r * 96:(r + 1) * 96],
                                         sb["s1"][:], start=True, stop=True)
                    ptMs[c] = ptM

                def emit_mm(c):
                    w = CHUNKS[c] * 96
                    o = COFF[c] * 96
                    mm = wpool.tile([96, w], F32, tag=f"mm{c}",
                                    name=f"mm{it}_{c}")
                    nc.vector.tensor_mul(
                        rview(mm[:], CHUNKS[c]),
                        pview(ptMs[c][:, :CHUNKS[c] * 128], CHUNKS[c]),
                        rview(sb["praw"][:, o:o + w], CHUNKS[c]))
                    mms[c] = mm

                def emit_fold(c):
                    # fold mm_c (CHUNKS[c]*96 wide) down to [96, 96]
                    cur = mms[c][:]
                    wcur = CHUNKS[c] * 96
                    lvl = 0
                    while wcur > 96:
                        half = wcur // 2
                        nxt = wpool.tile([96, half], F32,
                                         tag=f"q{c}_{lvl}",
                                         name=f"q{it}_{c}_{lvl}")
                        nc.vector.tensor_add(nxt[:], cur[:, :half],
                                             cur[:, half:])
                        cur = nxt[:]
                        wcur = half
                        lvl += 1
                    qs[c] = cur

                # PE order: Ag A0 Bg A1 B0 A2 B1 A3 B2 B3 (+warm stream)
                emit_A(0)
                ptMg = psM.tile([96, 8 * 128], F32, tag="ptM",
                                name=f"Mg{it}")
                nc.tensor.matmul(ptMg[:, 0:96], tsg[:], sb["g1"][:],
                                 start=True, stop=True)
                emit_ts(0)
                # gaussian message folded with base early: gb = 3*GhG + base
                mmg = wpool.tile([96, 96], F32, tag="mmg", name=f"mmg{it}")
                nc.vector.tensor_scalar_mul(mmg[:], ptMg[:, 0:96], 3.0)
                gb = wpool.tile([96, 96], F32, tag="gb", name=f"gb{it}")
                nc.vector.tensor_add(gb[:], mmg[:], basev[:])

                emit_A(1); emit_ts(1)
                emit_B(0); emit_mm(0)
                emit_fold(0)
                acc0 = wpool.tile([96, 96], F32, tag="acc0",
                                  name=f"acc0_{it}")
                nc.vector.tensor_add(acc0[:], qs[0][:], gb[:])
                emit_A(2); emit_ts(2)
                emit_B(1); emit_mm(1)
                emit_fold(1)
                acc1 = wpool.tile([96, 96], F32, tag="acc1",
                                  name=f"acc1_{it}")
                nc.vector.tensor_add(acc1[:], acc0[:], qs[1][:])
                emit_A(3); emit_ts(3)
                emit_B(2); emit_mm(2)
                emit_fold(2)
                acc2 = wpool.tile([96, 96], F32, tag="acc2",
                                  name=f"acc2_{it}")
                nc.vector.tensor_add(acc2[:], acc1[:], qs[2][:])
                emit_B(3); emit_mm(3)

                # back-to-back dummy matmuls cover the PE-idle tail so the
                # HAM clock-gate sees sustained activity (sparse pokes do
                # not prevent re-throttling; this held K=8/8 all-kernel)
                if it < NITER - 1:
                    dummies(17, f"warm{it}")

                emit_fold(3)
                logit = wpool.tile([96, 96], F32, tag="logit",
                                   name=f"logit{it}")
                nc.vector.tensor_add(logit[:], acc2[:], qs[3][:])

                if it == NITER - 1:
                    nc.sync.dma_start(out_t[:], logit[:])
                else:
                    h2 = cpool.tile([96, 96], F32, tag=f"h{it}",
                                    name=f"h{it}")
                    nc.scalar.activation(h2[:], logit[:], AF.Tanh, scale=0.5)
                    h = h2

    nc.compile()
    return nc


def _get_nc():
    if "nc" not in _CACHE:
        _CACHE["nc"] = _build()
    return _CACHE["nc"]


# ------------------------- entry point -------------------------

def kernel(image, mask):
    global LAST_RESULTS
    import os
    from concourse.bass_utils import run_bass_kernel_spmd

    shared = _host_constants(image, mask)
    nc = _get_nc()
    in_maps = [dict(shared) for _ in range(NCORES)]
    trace = bool(int(os.environ.get("KERNEL_TRACE", "0")))
    kw = {}
    if trace and os.environ.get("KERNEL_TRACE_ALL"):
        kw["trace_cores"] = list(range(NCORES))
        kw["stitch_traces"] = True
    try:
        res = run_bass_kernel_spmd(nc, in_maps, core_ids=list(range(NCORES)),
                                   trace=trace, **kw)
    except Exception:
        # one retry for transient device hiccups
        res = run_bass_kernel_spmd(nc, in_maps, core_ids=list(range(NCORES)),
                                   trace=trace, **kw)
    LAST_RESULTS = res
    logit_yx = res.results[0]["logit_out"]          # [y, x]
    pred = (logit_yx < 0).astype(np.float32).reshape(1, 1, H, W)
    return pred
